# revision 1
# baseline (speedup 1.0000x reference)
"""Trainium2 Bass kernel for nn_CrossPixelRefinement.

Reference computation (per point): scatter N=80000 sparse points into a
[B,2,H,W] grid, run conv1x1(2->8) -> conv7x1 -> conv1x7 -> gelu -> conv1x1(8->2)
+ residual, gather back at the same points, scale by s1.

Key insight: only the N scattered points are read back, and the conv chain's
receptive field is 7x7.  The three linear convs collapse into one [98 -> 8]
matrix M applied to each point's 7x7x2 neighborhood patch.

The grid is stored as overlapping vertical stripes (40 px wide, 32 px apart,
channel-last, y-major within a stripe) so a point's whole 7x7x2 patch is ONE
contiguous-window read: rows sit 80 elements apart inside a stripe, so a
560-element read starting at (stripe, y, xin) covers all 7 rows.  Only pixels
with (x+3) mod 32 < 8 also land in the previous stripe; the host appends
those as "ghost" duplicate scatter tokens.

Scatter uses the bulk dma_scatter_add path (0.34ns/desc instead of a 994ns
SWDGE setup per 128 points): the host builds one 256-byte one-hot row image
per token (the point's 2 bf16 values placed at offset&127 inside the 128-elem
grid row offset>>7) plus int16 row indices, and the device CCE-adds them into
the zeroed grid in 7 chunked calls (SSTRIDE is a multiple of 128 so rows
never straddle; pairs are even-aligned so they never split).  Gathers must
stay per-128-point indirect calls (one offset per SBUF partition per call).

Per core: zero grid, bulk scatter-add, compute coords/offsets on-device
(DVE), gather patches, PE-transpose each 128-point patch block, matmul with
M, gelu (ACT+DVE), 8->2 channel mix (DVE), add residual, scale s1, DMA out.

Sharding: data-parallel over batch; core c owns batches {2c, 2c+1}.  Conv
weights are folded host-side into M (tiny, replicated); per-point work is
on device.
"""

import os
import sys
from contextlib import ExitStack

import numpy as np

for _p in ("/opt/trn_rl_repo", "/root/.axon_site/_ro/trn_rl_repo"):
    if os.path.isdir(_p) and _p not in sys.path:
        sys.path.append(_p)

import ml_dtypes

import concourse.bass as bass
import concourse.bacc as bacc
import concourse.mybir as mybir
import concourse.tile as tile
from concourse.bass_utils import run_bass_kernel_spmd

F32 = mybir.dt.float32
BF16 = mybir.dt.bfloat16
I32 = mybir.dt.int32
I16 = mybir.dt.int16

# Problem geometry (fixed by the reference).
B, H, W, FS = 16, 640, 832, 2
N_CORES = 8
BPC = B // N_CORES            # batches per core
Hp = H + 6                    # halo rows
Hp2 = Hp + 2                  # + scrap rows (pads park at y=646; 648 rows
                              #   make SSTRIDE a multiple of 128)
Wp = W + 6                    # halo cols

# Stripe layout: stripes of 40 px (80 elems channel-last), stride 32 px.
# A gather window (7 px) always fits one stripe (overlap 8 >= 6); only pixels
# with (x+3) mod 32 < 8 also live in the previous stripe — those get host-side
# "ghost" duplicates in tail columns, so scatter is a single round.
TS = 32                       # stripe stride in px (exact /32 on device)
WS = 40                       # stripe width in px
NS = 27                       # stripes: scatter sp <= 26, gather sg <= 25
SROW = 2 * WS                 # elements per stripe row (80)
SSTRIDE = Hp2 * SROW          # elements per stripe (51840 = 405*128)
BSTRIDE = NS * SSTRIDE        # elements per batch  (1399680)
NG = BPC * BSTRIDE            # grid elements per core (2799360)
NROWS = NG // 128             # 256B rows for bulk scatter-add (21870 < 2^15)
FGATHER = 7 * SROW            # one patch read: 7 rows x 80 elems

P = 128                       # partitions
J = 80                        # gather columns (max real count 10100 <= 10240)
GJ = 21                       # ghost token slack (max ghost count 2488 <= 2688)
NPAD = P * J                  # real+pad point slots per core
NPAD_S = NPAD + P * GJ        # scatter tokens incl ghost duplicates (12928)

_cached = {"nc": None, "last_results": None}


def _build_nc(n_cores=N_CORES, repeat=1):
    """Build the Bass/Tile program (shared SPMD program for all cores)."""
    nc = bacc.Bacc("TRN2", target_bir_lowering=False, debug=False,
                   enable_asserts=False, num_devices=n_cores)

    pts_in = nc.declare_dram_parameter("pts", [P, 5 * J], F32, isOutput=False).ap()
    consts_in = nc.declare_dram_parameter("consts", [P, 32], F32, isOutput=False).ap()
    mmat_in = nc.declare_dram_parameter("mmat", [98, 8], BF16, isOutput=False).ap()
    ident_in = nc.declare_dram_parameter("ident", [P, P], BF16, isOutput=False).ap()
    # host-built scatter payload: 256B one-hot row images + wrapped row indices
    rowimg_in = nc.declare_dram_parameter("rowimg", [P, NPAD_S], BF16,
                                          isOutput=False).ap()
    sidx_in = nc.declare_dram_parameter("sidx", [P, NPAD_S // 16], I16,
                                        isOutput=False).ap()
    out_ext = nc.declare_dram_parameter("out", [P, 2 * J], F32, isOutput=True).ap()

    grid = nc.dram_tensor("grid", [NROWS, 128], BF16).ap()

    with tile.TileContext(nc) as tc:
        for _ in range(repeat):
            with ExitStack() as ctx:
                _kernel_body(ctx, tc, pts_in, consts_in, mmat_in, ident_in,
                             rowimg_in, sidx_in, out_ext, grid)
    nc.finalize()
    return nc


def _kernel_body(ctx, tc, pts_in, consts_in, mmat_in, ident_in,
                 rowimg_in, sidx_in, out_ext, grid):
    nc = tc.nc
    A = mybir.AluOpType

    const_pool = ctx.enter_context(tc.tile_pool(name="const", bufs=1))
    pts_pool = ctx.enter_context(tc.tile_pool(name="pts", bufs=1))
    big_pool = ctx.enter_context(tc.tile_pool(name="big", bufs=1))
    pt_pool = ctx.enter_context(tc.tile_pool(name="pt", bufs=3))
    psum_t = ctx.enter_context(tc.tile_pool(name="psum_t", bufs=3, space="PSUM"))
    psum_acc = ctx.enter_context(tc.tile_pool(name="psum_acc", bufs=1, space="PSUM"))

    # ---- load inputs -----------------------------------------------------
    pts = pts_pool.tile([P, 5 * J], F32)
    nc.sync.dma_start(pts[:], pts_in[:, :])
    fc0x, fc0y = pts[:, 0:J], pts[:, J:2 * J]
    fc1x, fc1y = pts[:, 2 * J:3 * J], pts[:, 3 * J:4 * J]
    bloc = pts[:, 4 * J:5 * J]

    rowimg = big_pool.tile([P, NPAD_S], BF16)
    nc.sync.dma_start(rowimg[:], rowimg_in[:, :])
    sidx = pts_pool.tile([P, NPAD_S // 16], I16)
    nc.sync.dma_start(sidx[:], sidx_in[:, :])

    consts = const_pool.tile([P, 32], F32)
    nc.sync.dma_start(consts[:], consts_in[:, :])

    mmat = const_pool.tile([98, 8], BF16)
    nc.sync.dma_start(mmat[:], mmat_in[:, :])
    ident = const_pool.tile([P, P], BF16)
    nc.sync.dma_start(ident[:], ident_in[:, :])

    # ---- zero the grid in DRAM (src/dst orders differ; zeros, so fine) ---
    ZC = 4096
    zt = big_pool.tile([P, ZC], BF16)
    nc.vector.memset(zt[:], 0.0)
    r = 0
    while r < NROWS:
        rows = min(ZC, NROWS - r)
        nc.sync.dma_start(grid[r:r + rows, :], zt[:, :rows * 128 // P])
        r += rows

    # ---- bulk scatter-add of host-built 256B one-hot row images ----------
    # chunked: one call's descriptors must fit the 128-slot SWDGE FIFO
    CH = 2048
    for t0 in range(0, NPAD_S, CH):
        n = min(CH, NPAD_S - t0)
        nc.gpsimd.dma_scatter_add(
            out_ap=grid[:, :],
            in_ap=rowimg[:, t0:t0 + n].rearrange("p (r e) -> p r e", e=128),
            idxs_ap=sidx[:, t0 // 16:(t0 + n) // 16],
            num_idxs=n,
            num_idxs_reg=n,
            elem_size=128,
        )

    # ---- per-point scalars via batch select ------------------------------
    # consts cols: 0..11 = rs0x0,rs0x1,rs0y0,rs0y1,rs1x0,rs1x1,rs1y0,rs1y1,
    # s1x0,s1x1,s1y0,s1y1; 12..13 = b4; 16..31 = 0.5*w4 flat.
    def sel(k):
        dif = pts_pool.tile([P, 1], F32, name=f"dif{k}")
        nc.vector.tensor_sub(dif[:], consts[:, k + 1:k + 2], consts[:, k:k + 1])
        out = pts_pool.tile([P, J], F32, name=f"sel{k}")
        nc.vector.scalar_tensor_tensor(
            out[:], bloc, dif[:, 0:1], consts[:, k:k + 1].to_broadcast([P, J]),
            op0=A.mult, op1=A.add)
        return out

    rs0x, rs0y = sel(0), sel(2)
    rs1x, rs1y = sel(4), sel(6)
    s1x, s1y = sel(8), sel(10)

    # ---- integer pixel coords (exact round-to-nearest-even) --------------
    def rounded_coord(fc, rs, name):
        t = pts_pool.tile([P, J], F32, name=f"t{name}")
        nc.vector.tensor_mul(t[:], fc, rs[:])
        ii = pts_pool.tile([P, J], I32, name=f"i{name}")
        nc.vector.tensor_scalar(ii[:], t[:], -0.5, None, A.add)
        f = pts_pool.tile([P, J], F32, name=f"f{name}")
        nc.vector.tensor_copy(f[:], ii[:])
        return f

    ixf = rounded_coord(fc0x, rs0x, "x")   # in [0, W) (pads at 0)
    iyf = rounded_coord(fc0y, rs0y, "y")   # in [0, H) (pads at H+3)

    # ---- point values (residual + scatter payload) -----------------------
    vx = pts_pool.tile([P, J], F32)
    nc.vector.tensor_mul(vx[:], fc1x, rs1x[:])
    vy = pts_pool.tile([P, J], F32)
    nc.vector.tensor_mul(vy[:], fc1y, rs1y[:])

    # ---- flat element offsets (all integer-valued f32, < 2^23: exact) ----
    def floor32(src, name, cols):
        # floor(src/32) for integer-valued src >= 0 (x*0.03125 is exact;
        # frac is k/32, and 31/64 keeps RNE inside (m-1/2, m+1/2))
        z = pts_pool.tile([P, cols], F32, name=f"z{name}")
        nc.vector.tensor_scalar(z[:], src, 0.03125, None, A.mult)
        ii = pts_pool.tile([P, cols], I32, name=f"zi{name}")
        nc.vector.tensor_scalar(ii[:], z[:], -0.484375, None, A.add)
        f = pts_pool.tile([P, cols], F32, name=f"zf{name}")
        nc.vector.tensor_copy(f[:], ii[:])
        return f

    # gather: window starts at padded (iy, ix) -> stripe sg = floor(ix/32)
    sg = floor32(ixf[:, :J], "sg", J)
    g1 = pts_pool.tile([P, J], F32)
    nc.vector.tensor_scalar(g1[:], iyf[:, :J], float(SROW), None, A.mult)
    g2 = pts_pool.tile([P, J], F32)
    nc.vector.scalar_tensor_tensor(g2[:], bloc[:, :J], float(BSTRIDE), g1[:],
                                   op0=A.mult, op1=A.add)
    g3t = pts_pool.tile([P, J], F32)
    nc.vector.scalar_tensor_tensor(g3t[:], ixf[:, :J], 2.0, g2[:],
                                   op0=A.mult, op1=A.add)
    gofff = pts_pool.tile([P, J], F32)
    nc.vector.scalar_tensor_tensor(gofff[:], sg[:], float(SSTRIDE - 64), g3t[:],
                                   op0=A.mult, op1=A.add)
    goff = pts_pool.tile([P, J], I32)
    nc.vector.tensor_copy(goff[:], gofff[:])

    # ---- gather + conv chain, chunk-pipelined ----------------------------
    patches = big_pool.tile([P, FGATHER * J], BF16)
    pat4 = patches[:, :].rearrange("p (j k e) -> p j k e", k=7, e=SROW)

    n_groups = (J + 63) // 64
    group_tiles = []
    for gi in range(n_groups):
        cols = min(64, J - gi * 64) * 8
        group_tiles.append(psum_acc.tile([P, cols], F32, name=f"grp{gi}"))

    for j in range(J):
        nc.gpsimd.indirect_dma_start(
            out=patches[:, j * FGATHER:(j + 1) * FGATHER],
            out_offset=None,
            in_=grid[:, :],
            in_offset=bass.IndirectOffsetOnAxis(ap=goff[:, j:j + 1], axis=1),
        )
        blk = pat4[:, j, :, 0:14]          # [128, 7, 14] strided view
        cmp = pt_pool.tile([P, 98], BF16, name="cmp", tag="cmp")
        nc.vector.tensor_copy(cmp[:, :].rearrange("p (k e) -> p k e", e=14), blk)
        ptp = psum_t.tile([98, P], BF16, name="ptp", tag="ptp")
        nc.tensor.transpose(ptp[:], cmp[:], ident[:])
        pt = pt_pool.tile([98, P], BF16, name="pt", tag="pt")
        nc.vector.tensor_copy(pt[:], ptp[:])
        gi, lj = j // 64, j % 64
        nc.tensor.matmul(group_tiles[gi][:, lj * 8:(lj + 1) * 8],
                         lhsT=pt[:], rhs=mmat[:], start=True, stop=True)

    # ---- gelu (tanh approx) from primitives ------------------------------
    # g = 2*gelu(t) = (1 + tanh(0.79788456*(t + 0.044715 t^3))) * t
    # the 0.5 is folded into w4 host-side.
    g4 = big_pool.tile([P, 8 * J], F32)
    for gi in range(n_groups):
        lo = gi * 512
        cols = group_tiles[gi].shape[1]
        t = pts_pool.tile([P, cols], F32, name=f"gelu_t{gi}", tag="gelu_t")
        nc.vector.tensor_copy(t[:], group_tiles[gi][:])
        u = pts_pool.tile([P, cols], F32, name=f"gelu_u{gi}", tag="gelu_u")
        nc.vector.tensor_mul(u[:], t[:], t[:])
        w = pts_pool.tile([P, cols], F32, name=f"gelu_w{gi}", tag="gelu_w")
        nc.vector.tensor_mul(w[:], u[:], t[:])
        v = pts_pool.tile([P, cols], F32, name=f"gelu_v{gi}", tag="gelu_v")
        nc.vector.scalar_tensor_tensor(v[:], w[:], 0.044715, t[:],
                                       op0=A.mult, op1=A.add)
        z = pts_pool.tile([P, cols], F32, name=f"gelu_z{gi}", tag="gelu_z")
        nc.scalar.activation(z[:], v[:], mybir.ActivationFunctionType.Tanh,
                             bias=0.0, scale=0.7978845608028654)
        nc.vector.scalar_tensor_tensor(g4[:, lo:lo + cols], z[:], 1.0, t[:],
                                       op0=A.add, op1=A.mult)

    # ---- conv4: 8 -> 2 channel mix along free dim ------------------------
    g43 = g4[:, :].rearrange("p (j m) -> p j m", m=8)
    out_t = pts_pool.tile([P, 2 * J], F32)
    o3 = out_t[:, :].rearrange("p (j c) -> p j c", c=2)
    for c, (vv, ss) in enumerate(((vx[:, :J], s1x[:, :J]),
                                  (vy[:, :J], s1y[:, :J]))):
        acc = pts_pool.tile([P, J], F32, name=f"acc{c}")
        nc.vector.tensor_scalar(acc[:], g43[:, :, 0],
                                consts[:, 16 + 8 * c:17 + 8 * c],
                                None, A.mult)
        for m in range(1, 8):
            nc.vector.scalar_tensor_tensor(
                acc[:], g43[:, :, m], consts[:, 16 + 8 * c + m:17 + 8 * c + m],
                acc[:], op0=A.mult, op1=A.add)
        # h = acc + b4_c + vals_c ; out = h * s1_c
        h = pts_pool.tile([P, J], F32, name=f"h{c}")
        nc.vector.scalar_tensor_tensor(h[:], acc[:], consts[:, 12 + c:13 + c],
                                       vv, op0=A.add, op1=A.add)
        nc.vector.tensor_mul(o3[:, :, c], h[:], ss)

    nc.sync.dma_start(out_ext[:, :], out_t[:])


def _host_prep(inputs):
    """Shard + lay out inputs per core; returns in_maps and unperm info."""
    fc0 = np.ascontiguousarray(inputs["fine_coord_0"], dtype=np.float32)
    fc1 = np.ascontiguousarray(inputs["fine_coord_1"], dtype=np.float32)
    b_idx = np.ascontiguousarray(inputs["b_idx_it"]).astype(np.int64)
    scale0 = np.ascontiguousarray(inputs["scale0"], dtype=np.float32)
    scale1 = np.ascontiguousarray(inputs["scale1"], dtype=np.float32)
    w1 = np.asarray(inputs["w1"], dtype=np.float32)[:, :, 0, 0]      # [8,2]
    w2 = np.asarray(inputs["w2"], dtype=np.float32)[:, :, :, 0]      # [8,8,7]
    w3 = np.asarray(inputs["w3"], dtype=np.float32)[:, :, 0, :]      # [8,8,7]
    w4 = np.asarray(inputs["w4"], dtype=np.float32)[:, :, 0, 0]      # [2,8]
    b4 = np.asarray(inputs["b4"], dtype=np.float32)

    # fold conv1/conv2/conv3 into M [98, 8] (patch layout (y, x, c) -> out ch)
    M64 = np.einsum("oax,aby,bc->yxco", w3.astype(np.float64),
                    w2.astype(np.float64), w1.astype(np.float64))
    mmat = M64.reshape(98, 8).astype(np.float32).astype(ml_dtypes.bfloat16)

    s0 = (scale0 * FS).astype(np.float32)       # [B,2]
    s1 = (scale1 * FS).astype(np.float32)
    rs0 = (1.0 / s0.astype(np.float64)).astype(np.float32)
    rs1 = (1.0 / s1.astype(np.float64)).astype(np.float32)

    ident = np.eye(P, dtype=ml_dtypes.bfloat16)

    # integer pixel coords exactly as the device computes them (f32 RNE)
    ix_all = np.rint(fc0[:, 0] * rs0[b_idx, 0] - np.float32(0.5)).astype(np.int64)
    iy_all = np.rint(fc0[:, 1] * rs0[b_idx, 1] - np.float32(0.5)).astype(np.int64)
    # scatter values, f32 then bf16 RNE — matches the device value pipeline
    vx_all = (fc1[:, 0] * rs1[b_idx, 0]).astype(ml_dtypes.bfloat16)
    vy_all = (fc1[:, 1] * rs1[b_idx, 1]).astype(ml_dtypes.bfloat16)

    in_maps = []
    sels = []
    for c in range(N_CORES):
        b0 = BPC * c
        sel = np.nonzero((b_idx >= b0) & (b_idx < b0 + BPC))[0]
        cnt = len(sel)
        if cnt > NPAD:
            raise ValueError(f"core {c}: {cnt} points > NPAD={NPAD}")
        sels.append(sel)

        # flat scatter offsets; ghosts (overlap pixels) go to stripe sp-1
        xc = ix_all[sel] + 3
        sp = xc >> 5
        off = ((b_idx[sel] - b0) * BSTRIDE + sp * SSTRIDE
               + (iy_all[sel] + 3) * SROW + 2 * (xc - (sp << 5)))
        gmask = (sp >= 1) & ((xc - (sp << 5)) < 8)
        gcnt = int(gmask.sum())
        if gcnt > P * GJ:
            raise ValueError(f"core {c}: {gcnt} ghosts > {P * GJ}")
        off_g = off[gmask] - SSTRIDE + 2 * TS

        # one-hot 256B row images + row indices (pads: index 0, zero row)
        rows = np.zeros((NPAD_S, 128), ml_dtypes.bfloat16)
        rid = np.zeros(NPAD_S, np.int16)
        for lo, o, idxs in ((0, off, sel), (NPAD, off_g, sel[gmask])):
            t = lo + np.arange(len(o))
            pos = (o & 127).astype(np.int64)
            rows[t, pos] = vx_all[idxs]
            rows[t, pos + 1] = vy_all[idxs]
            rid[t] = (o >> 7).astype(np.int16)
        # token t -> payload partition t%128 row t//128; index slot t%16, t//16
        rowimg = rows.reshape(NPAD_S // 128, 128, 128).transpose(1, 0, 2)
        rowimg = np.ascontiguousarray(rowimg.reshape(128, NPAD_S))
        sidx = np.ascontiguousarray(
            np.tile(rid.reshape(NPAD_S // 16, 16).T, (8, 1)))

        pts = np.zeros((5, NPAD), np.float32)
        # default all slots to the pad point (ix, iy) = (0, H+3) -> scrap row
        pts[0, :] = 0.5 * s0[b0, 0]
        pts[1, :] = (H + 3.5) * s0[b0, 1]
        pts[0, :cnt] = fc0[sel, 0]
        pts[1, :cnt] = fc0[sel, 1]
        pts[2, :cnt] = fc1[sel, 0]
        pts[3, :cnt] = fc1[sel, 1]
        pts[4, :cnt] = (b_idx[sel] - b0).astype(np.float32)
        # device tile layout [P, 5*J], partition-minor: point i = j*P + p
        pts_t = np.concatenate([pts[q].reshape(J, P).T for q in range(5)],
                               axis=1)

        sc = np.zeros(32, np.float32)
        sc[0:2] = rs0[b0:b0 + 2, 0]
        sc[2:4] = rs0[b0:b0 + 2, 1]
        sc[4:6] = rs1[b0:b0 + 2, 0]
        sc[6:8] = rs1[b0:b0 + 2, 1]
        sc[8:10] = s1[b0:b0 + 2, 0]
        sc[10:12] = s1[b0:b0 + 2, 1]
        sc[12:14] = b4
        sc[16:24] = 0.5 * w4[0]   # 0.5 from the gelu formula folded in
        sc[24:32] = 0.5 * w4[1]
        consts = np.broadcast_to(sc, (P, 32)).copy()

        in_maps.append({
            "pts": pts_t,
            "consts": consts,
            "mmat": np.ascontiguousarray(mmat),
            "ident": ident,
            "rowimg": rowimg,
            "sidx": sidx,
        })
    return in_maps, sels


def kernel(**inputs) -> np.ndarray:
    if _cached["nc"] is None:
        _cached["nc"] = _build_nc()
    nc = _cached["nc"]

    in_maps, sels = _host_prep(inputs)
    res = run_bass_kernel_spmd(nc, in_maps, list(range(N_CORES)))
    _cached["last_results"] = res

    n = inputs["fine_coord_0"].shape[0]
    out = np.zeros((n, 2), np.float32)
    for c in range(N_CORES):
        oc = np.asarray(res.results[c]["out"]).reshape(P, J, 2)
        oc = oc.transpose(1, 0, 2).reshape(NPAD, 2)   # point i = j*P + p
        out[sels[c]] = oc[:len(sels[c])]
    return out

